# revision 18
# baseline (speedup 1.0000x reference)
"""Trainium2 Bass kernel for nn_Net1 (gnn_message_passing), 8-core SPMD.

Sharding: data-parallel over batch B=8 — core c computes batch element c.
Per core the dominant work is streaming its 64 MiB link_mtx slice once from
HBM through a float32r matmul (memory-bound); everything else (tree-scan
attention, final projection) is interleaved into the DMA shadow.

Device layout choices made on the host (input staging):
  - link_mtx[c] is passed TRANSPOSED (linkT [m, n]) so the contraction
    index m lands on SBUF partitions and DMA stays fully contiguous.
  - rel_table is passed transposed (relT [64, 4096]) to feed the on-device
    P = rel_table @ graph_weight precompute without PE transposes.
  - The tree-scan embedding gathers (tiny: a few hundred KB of table rows)
    are resolved on the host into per-step packed operands; all tree
    compute (QKV projections, attention, softmax, reductions) runs on
    device.

Math identity used for the graph part:
  relu(link @ rel @ W + b)  ==  relu(link @ P + b),  P = rel @ W
  pooled = mean_n relu(...)  — accumulated per 512-col PSUM bank with a
  fused relu+bias+row-sum activation.
"""

import numpy as np

import concourse.bass as bass
import concourse.mybir as mybir
from concourse.tile import TileContext
from concourse.bass_utils import run_bass_kernel_spmd

F32 = mybir.dt.float32
F32R = mybir.dt.float32r
AF = mybir.ActivationFunctionType
AX = mybir.AxisListType

B, T, D, C = 8, 8, 16, 2
R, NCOL, A = 4096, 20000, 4096
E, G = 64, 64
N_CORES = 8

FLOAT_MAX = 3.4e38
FLOAT_MIN = -3.4e38

# graph stream tiling: 6 PSUM banks for n-columns 0:3072, then 2 banks for
# 3072:4096 — leaves 2 banks free for the tree scan / prologue / epilogue.
NB_A = 5          # n-blocks (512 wide) in group A
NB_B = 3          # n-blocks in group B
MCHUNKS = R // 128


def _split_waits(nc, maxw=1):
    """walrus's CTRL lowering rejects >1 sync wait on an instruction; hoist
    extras onto preceding same-engine InstNoOps (program order preserves
    the wait semantics)."""
    n = 0
    for f in nc.m.functions:
        for blk in f.blocks:
            new_insts = []
            for inst in blk.instructions:
                si = inst.sync_info
                waits = list(si.on_wait) if (si and si.on_wait) else []
                if len(waits) > maxw:
                    extra, keep = waits[:-maxw], waits[-maxw:]
                    while extra:
                        chunk, extra = extra[:maxw], extra[maxw:]
                        nop = mybir.InstNoOp(
                            name=f"waitsplit-{inst.name}-{n}", ins=[], outs=[])
                        nop.engine = inst.engine
                        nop.sync_info = mybir.SyncInfo(on_wait=chunk, on_update=[])
                        new_insts.append(nop)
                        n += 1
                    inst.sync_info = mybir.SyncInfo(
                        on_wait=keep,
                        on_update=list(si.on_update) if si.on_update else [])
                new_insts.append(inst)
            blk.instructions[:] = new_insts
    return n


def _build_nc():
    nc = bass.Bass()

    linkT = nc.dram_tensor("linkt", [R, R], F32R, kind="ExternalInput")
    pmat = nc.dram_tensor("pmat", [128, MCHUNKS * G], F32R, kind="ExternalInput")
    gb = nc.dram_tensor("gb", [G, 1], F32, kind="ExternalInput")
    wkqt = nc.dram_tensor("wkqt", [E, E], F32, kind="ExternalInput")
    wv = nc.dram_tensor("wv", [E, E], F32, kind="ExternalInput")
    fcw = nc.dram_tensor("fcw", [G + E, A], F32R, kind="ExternalInput")
    amask = nc.dram_tensor("amask", [1, A], F32, kind="ExternalInput")
    xstat = nc.dram_tensor("xstat", [E, D * 24], F32, kind="ExternalInput")
    xinit = nc.dram_tensor("xinit", [E, T], F32, kind="ExternalInput")
    m01 = nc.dram_tensor("m01", [4 * T, 4 * T], F32, kind="ExternalInput")
    gsel = nc.dram_tensor("gsel", [4 * T, T], F32, kind="ExternalInput")
    out = nc.dram_tensor("out", [1, A], F32, kind="ExternalOutput")

    with TileContext(nc) as tc:
        with (
            tc.tile_pool(name="one", bufs=1) as one,      # singles / constants
            tc.tile_pool(name="lnk", bufs=8) as lnk,      # link stream tiles
            tc.tile_pool(name="sml", bufs=3) as sml,      # small working tiles
            tc.tile_pool(name="gps", bufs=NB_A, space="PSUM") as gps,
            tc.tile_pool(name="tps", bufs=3, space="PSUM") as tps,
        ):
            # ---- load constants / small inputs ----
            gb_s = one.tile([G, 1], F32, name="gb_s", tag="gb_s")
            nc.scalar.dma_start(out=gb_s, in_=gb[:, :])
            wkqt_s = one.tile([E, E], F32, name="wkqt_s", tag="wkqt_s")
            nc.scalar.dma_start(out=wkqt_s, in_=wkqt[:, :])
            wv_s = one.tile([E, E], F32, name="wv_s", tag="wv_s")
            nc.scalar.dma_start(out=wv_s, in_=wv[:, :])
            xstat_s = one.tile([E, D * 24], F32, name="xstat_s", tag="xstat_s")
            nc.scalar.dma_start(out=xstat_s, in_=xstat[:, :])
            m01_s = one.tile([4 * T, 4 * T], F32, name="m01_s", tag="m01_s")
            nc.scalar.dma_start(out=m01_s, in_=m01[:, :])
            gsel_s = one.tile([4 * T, T], F32, name="gsel_s", tag="gsel_s")
            nc.scalar.dma_start(out=gsel_s, in_=gsel[:, :])
            ones_s = one.tile([4 * T, 1], F32, name="ones_s", tag="ones_s")
            nc.vector.memset(ones_s, 1.0)

            # inf_mask = clip(log(action_mask)) — [1, A] ops are lane-serial
            # (~4us each), so run them here where the link stream hides them
            lm_s = one.tile([1, A], F32, name="lm_s", tag="lm_s")
            nc.scalar.dma_start(out=lm_s, in_=amask[:, :])
            nc.scalar.activation(lm_s, lm_s, AF.Ln)
            nc.vector.tensor_scalar_max(lm_s, lm_s, FLOAT_MIN)
            nc.vector.tensor_scalar_min(lm_s, lm_s, FLOAT_MAX)

            # ---- P = rel_table @ graph_weight (host-folded), [m, 32*64] ----
            p_s = one.tile([128, MCHUNKS * G], F32R, name="p_s", tag="p_s")
            nc.scalar.dma_start(out=p_s, in_=pmat[:, :])

            # ---- tree scan, one step at a time (interleaved into the
            #      graph stream's PE program below) ----
            x_cur = sml.tile([E, 4 * T], F32, name="x0", tag="xtile")
            nc.scalar.dma_start(out=x_cur[:, 0:T], in_=xinit[:, :])
            nc.vector.tensor_copy(x_cur[:, T:4 * T], xstat_s[:, 0:24])
            prev_final = one.tile([E, T], F32, name="prev_final",
                                  tag="prev_final")

            def tree_steps():
                # 4 yields per step: each PE sub-part's producers get a
                # ~1-2 graph-m-chunk head start, so the in-order PE queue
                # never stalls on the tree chain
                nonlocal x_cur
                for d in range(D):
                    # A = (Wk Wq^T) X  — folded kq product (host input)
                    a_p = tps.tile([E, 4 * T], F32, name="a_p", tag="tp")
                    nc.tensor.matmul(a_p, wkqt_s, x_cur, start=True, stop=True)
                    a_s = sml.tile([E, 4 * T], F32, name="a_s", tag="a_s")
                    nc.vector.tensor_copy(a_s, a_p)
                    v_p = tps.tile([4 * T, E], F32, name="v_p", tag="tp")
                    nc.tensor.matmul(v_p, x_cur, wv_s, start=True, stop=True)
                    v_s = sml.tile([4 * T, E], F32, name="v_s", tag="v_s")
                    nc.vector.tensor_copy(v_s, v_p)
                    yield
                    # S_T = X^T A  — S_T[k, q] = K[k]·Q[q]
                    s_p = tps.tile([4 * T, 4 * T], F32, name="s_p", tag="tp")
                    nc.tensor.matmul(s_p, x_cur, a_s, start=True, stop=True)
                    exp_s = sml.tile([4 * T, 4 * T], F32, name="exp_s",
                                     tag="exp_s")
                    nc.scalar.activation(exp_s, s_p, AF.Exp, scale=1.0 / E)
                    expm_s = sml.tile([4 * T, 4 * T], F32, name="expm_s",
                                      tag="expm_s")
                    nc.vector.tensor_mul(expm_s, exp_s, m01_s)
                    yield
                    rs_p = tps.tile([4 * T, 1], F32, name="rs_p", tag="tp")
                    nc.tensor.matmul(rs_p, expm_s, ones_s, start=True,
                                     stop=True)
                    rec_s = sml.tile([4 * T, 1], F32, name="rec_s",
                                     tag="rec_s")
                    nc.vector.reciprocal(rec_s, rs_p)
                    ou_p = tps.tile([4 * T, E], F32, name="ou_p", tag="tp")
                    nc.tensor.matmul(ou_p, expm_s, v_s, start=True, stop=True)
                    no_s = sml.tile([4 * T, E], F32, name="no_s", tag="no_s")
                    nc.vector.tensor_scalar_mul(no_s, ou_p, rec_s)
                    yield
                    nx_p = tps.tile([E, T], F32, name="nx_p", tag="tp")
                    nc.tensor.matmul(nx_p, no_s, gsel_s, start=True, stop=True)
                    if d + 1 < D:
                        x_nxt = sml.tile([E, 4 * T], F32, name="xn",
                                         tag="xtile")
                        nc.vector.tensor_copy(x_nxt[:, 0:T], nx_p)
                        nc.vector.tensor_copy(
                            x_nxt[:, T:4 * T],
                            xstat_s[:, (d + 1) * 24:(d + 2) * 24])
                        x_cur = x_nxt
                    else:
                        nc.vector.tensor_copy(prev_final, nx_p)
                    yield

            stepper = tree_steps()

            # ---- graph stream ----
            part_s = one.tile([G, NB_A + NB_B], F32, name="part_s",
                              tag="part_s")

            def stream_group(n0, nblocks, interleave):
                ps = []
                for j in range(nblocks):
                    pt = gps.tile([G, 512], F32, name=f"gp{j}", tag="gp")
                    ps.append(pt)
                width = nblocks * 512
                for mi in range(MCHUNKS):
                    lt = lnk.tile([128, width], F32R, name="lt", tag="lt")
                    nc.sync.dma_start(
                        out=lt,
                        in_=linkT[mi * 128:(mi + 1) * 128, n0:n0 + width])
                    lhs = p_s[:, mi * G:(mi + 1) * G]
                    for j in range(nblocks):
                        nc.tensor.matmul(
                            ps[j], lhs,
                            lt[:, j * 512:(j + 1) * 512],
                            start=(mi == 0), stop=(mi == MCHUNKS - 1))
                    if interleave:
                        next(stepper, None)
                return ps

            ps_a = stream_group(0, NB_A, True)
            for j in range(NB_A):
                rsc = sml.tile([G, 512], F32, name="rsc", tag="rsc")
                nc.scalar.activation(rsc, ps_a[j], AF.Relu, bias=gb_s,
                                     scale=1.0, accum_out=part_s[:, j:j + 1])
            # final-matmul weights: loaded here so the transfer hides under
            # the group-B stream instead of blocking the head of the kernel
            fcw_s = one.tile([G + E, A], F32R, name="fcw_s", tag="fcw_s")
            nc.scalar.dma_start(out=fcw_s, in_=fcw[:, :])
            ps_b = stream_group(NB_A * 512, NB_B, True)
            for _ in stepper:  # safety: finish any untraced tree steps
                pass
            for j in range(NB_B):
                rsc = sml.tile([G, 512], F32, name="rsc", tag="rsc")
                nc.scalar.activation(rsc, ps_b[j], AF.Relu, bias=gb_s,
                                     scale=1.0,
                                     accum_out=part_s[:, NB_A + j:NB_A + j + 1])

            pooled_f = one.tile([G, 1], F32, name="pooled_f", tag="pooled_f")
            nc.vector.reduce_sum(pooled_f, part_s, axis=AX.X)
            pooled_s = one.tile([G, 1], F32R, name="pooled_s", tag="pooled_s")
            nc.vector.tensor_scalar_mul(pooled_s, pooled_f, 1.0 / R)

            # tree_emb = sum_t prev_final
            temb_f = one.tile([E, 1], F32, name="temb_f", tag="temb_f")
            nc.vector.reduce_sum(temb_f, prev_final, axis=AX.X)
            temb_s = one.tile([E, 1], F32R, name="temb_s", tag="temb_s")
            nc.vector.tensor_copy(temb_s, temb_f)

            # logits = [pooled; tree_emb] @ fc_w + inf_mask
            feat_s = one.tile([G + E, 1], F32R, name="feat_s", tag="feat_s")
            nc.sync.dma_start(out=feat_s[0:G, :], in_=pooled_s)
            nc.sync.dma_start(out=feat_s[G:G + E, :], in_=temb_s)
            out_s = one.tile([1, A], F32, name="out_s", tag="out_s")
            for j in range(A // 512):
                lg_p = tps.tile([1, 512], F32, name="lg_p", tag="tp")
                nc.tensor.matmul(lg_p, feat_s,
                                 fcw_s[:, j * 512:(j + 1) * 512],
                                 start=True, stop=True)
                nc.vector.tensor_add(out_s[:, j * 512:(j + 1) * 512], lg_p,
                                     lm_s[:, j * 512:(j + 1) * 512])
            nc.sync.dma_start(out=out[:, :], in_=out_s)

    _split_waits(nc)
    return nc


_NC_CACHE = None


def _get_nc():
    global _NC_CACHE
    if _NC_CACHE is None:
        _NC_CACHE = _build_nc()
    return _NC_CACHE


def _prepare_in_maps(inputs):
    link_mtx = np.ascontiguousarray(np.asarray(inputs["link_mtx"], np.float32))
    action_mask = np.asarray(inputs["action_mask"], np.float32)
    col = np.asarray(inputs["col_table"], np.float32)
    rel = np.asarray(inputs["rel_table"], np.float32)
    gw = np.ascontiguousarray(np.asarray(inputs["graph_weight"], np.float32))
    gb = np.asarray(inputs["graph_bias"], np.float32).reshape(G, 1)
    wk = np.ascontiguousarray(np.asarray(inputs["Wk"], np.float32))
    wq = np.ascontiguousarray(np.asarray(inputs["Wq"], np.float32))
    wv = np.ascontiguousarray(np.asarray(inputs["Wv"], np.float32))
    fcw = np.ascontiguousarray(np.asarray(inputs["fc_w"], np.float32))
    tid = np.asarray(inputs["table_ids"])
    lci = np.asarray(inputs["l_col_ids"])
    rci = np.asarray(inputs["r_col_ids"])

    gb = np.ascontiguousarray(gb)
    pmat = np.ascontiguousarray((rel @ gw).reshape(MCHUNKS, 128, G)
                                .transpose(1, 0, 2).reshape(128, MCHUNKS * G))
    wkqt = np.ascontiguousarray(wq @ wk.T)

    i = np.arange(4 * T)
    m01 = (i[:, None] % T == i[None, :] % T).astype(np.float32)
    gsel = (i[:, None] % T == np.arange(T)[None, :]).astype(np.float32)

    in_maps = []
    for c in range(N_CORES):
        linkT = np.ascontiguousarray(link_mtx[c].T)
        # host-side embedding gathers for the tree scan (layout prep)
        rtabs = rel[tid[c, :, 1:]]            # [T, D, E]
        lcols = col[lci[c]].mean(axis=-2)     # [T, D, E]
        rcols = col[rci[c]].mean(axis=-2)     # [T, D, E]
        xstat = np.empty((E, D, 24), np.float32)
        xstat[:, :, 0:8] = lcols.transpose(2, 1, 0)
        xstat[:, :, 8:16] = rcols.transpose(2, 1, 0)
        xstat[:, :, 16:24] = rtabs.transpose(2, 1, 0)
        xinit = np.ascontiguousarray(rel[tid[c, :, 0]].T)  # [E, T]
        in_maps.append({
            "linkt": linkT,
            "pmat": pmat,
            "gb": gb,
            "wkqt": wkqt,
            "wv": wv,
            "fcw": fcw,
            "amask": np.ascontiguousarray(action_mask[c:c + 1]),
            "xstat": np.ascontiguousarray(xstat.reshape(E, D * 24)),
            "xinit": xinit,
            "m01": m01,
            "gsel": gsel,
        })
    return in_maps


def _run(inputs, trace=False):
    nc = _get_nc()
    in_maps = _prepare_in_maps(inputs)
    res = run_bass_kernel_spmd(nc, in_maps, core_ids=list(range(N_CORES)),
                               trace=trace)
    out = np.stack([res.results[c]["out"].reshape(A) for c in range(N_CORES)])
    return out.astype(np.float32), res


def kernel(**inputs) -> np.ndarray:
    out, _ = _run(inputs, trace=False)
    return out


def kernel_traced(**inputs):
    out, res = _run(inputs, trace=True)
    return out, res


# revision 20
# speedup vs baseline: 1.2026x; 1.2026x over previous
"""Trainium2 Bass kernel for nn_Net1 (gnn_message_passing), 8-core SPMD.

Sharding: data-parallel over batch B=8 — core c computes batch element c.
Per core the dominant work is streaming its 64 MiB link_mtx slice once from
HBM through a float32r matmul (memory-bound); everything else (tree-scan
attention, final projection) is interleaved into the DMA shadow.

Device layout choices made on the host (input staging):
  - link_mtx[c] is passed TRANSPOSED (linkT [m, n]) so the contraction
    index m lands on SBUF partitions and DMA stays fully contiguous.
  - rel_table is passed transposed (relT [64, 4096]) to feed the on-device
    P = rel_table @ graph_weight precompute without PE transposes.
  - The tree-scan embedding gathers (tiny: a few hundred KB of table rows)
    are resolved on the host into per-step packed operands; all tree
    compute (QKV projections, attention, softmax, reductions) runs on
    device.

Math identity used for the graph part:
  relu(link @ rel @ W + b)  ==  relu(link @ P + b),  P = rel @ W
  pooled = mean_n relu(...)  — accumulated per 512-col PSUM bank with a
  fused relu+bias+row-sum activation.
"""

import numpy as np

import concourse.bass as bass
import concourse.mybir as mybir
from concourse.tile import TileContext
from concourse.bass_utils import run_bass_kernel_spmd

F32 = mybir.dt.float32
F32R = mybir.dt.float32r
AF = mybir.ActivationFunctionType
AX = mybir.AxisListType

B, T, D, C = 8, 8, 16, 2
R, NCOL, A = 4096, 20000, 4096
E, G = 64, 64
N_CORES = 8

FLOAT_MAX = 3.4e38
FLOAT_MIN = -3.4e38

# graph stream tiling: 6 PSUM banks for n-columns 0:3072, then 2 banks for
# 3072:4096 — leaves 2 banks free for the tree scan / prologue / epilogue.
NB_A = 5          # n-blocks (512 wide) in group A
NB_B = 3          # n-blocks in group B
MCHUNKS = R // 128


def _split_waits(nc, maxw=1):
    """walrus's CTRL lowering rejects >1 sync wait on an instruction; hoist
    extras onto preceding same-engine InstNoOps (program order preserves
    the wait semantics)."""
    n = 0
    for f in nc.m.functions:
        for blk in f.blocks:
            new_insts = []
            for inst in blk.instructions:
                si = inst.sync_info
                waits = list(si.on_wait) if (si and si.on_wait) else []
                if len(waits) > maxw:
                    extra, keep = waits[:-maxw], waits[-maxw:]
                    while extra:
                        chunk, extra = extra[:maxw], extra[maxw:]
                        nop = mybir.InstNoOp(
                            name=f"waitsplit-{inst.name}-{n}", ins=[], outs=[])
                        nop.engine = inst.engine
                        nop.sync_info = mybir.SyncInfo(on_wait=chunk, on_update=[])
                        new_insts.append(nop)
                        n += 1
                    inst.sync_info = mybir.SyncInfo(
                        on_wait=keep,
                        on_update=list(si.on_update) if si.on_update else [])
                new_insts.append(inst)
            blk.instructions[:] = new_insts
    return n


def _build_nc():
    nc = bass.Bass()

    linkT = nc.dram_tensor("linkt", [R, R], F32R, kind="ExternalInput")
    pmat = nc.dram_tensor("pmat", [128, MCHUNKS * G], F32R, kind="ExternalInput")
    gb = nc.dram_tensor("gb", [G, 1], F32, kind="ExternalInput")
    wkqt = nc.dram_tensor("wkqt", [E, E], F32, kind="ExternalInput")
    wv = nc.dram_tensor("wv", [E, E], F32, kind="ExternalInput")
    fcw = nc.dram_tensor("fcw", [G + E, A], F32R, kind="ExternalInput")
    amask = nc.dram_tensor("amask", [1, A], F32, kind="ExternalInput")
    xstat = nc.dram_tensor("xstat", [E, D * 24], F32, kind="ExternalInput")
    xinit = nc.dram_tensor("xinit", [E, T], F32, kind="ExternalInput")
    m01 = nc.dram_tensor("m01", [4 * T, 4 * T], F32, kind="ExternalInput")
    gsel = nc.dram_tensor("gsel", [4 * T, T], F32, kind="ExternalInput")
    out = nc.dram_tensor("out", [1, A], F32, kind="ExternalOutput")

    with TileContext(nc) as tc:
        with (
            tc.tile_pool(name="one", bufs=1) as one,      # singles / constants
            tc.tile_pool(name="lnk", bufs=8) as lnk,      # link stream tiles
            tc.tile_pool(name="sml", bufs=3) as sml,      # small working tiles
            tc.tile_pool(name="gps", bufs=NB_A, space="PSUM") as gps,
            tc.tile_pool(name="tps", bufs=3, space="PSUM") as tps,
        ):
            # ---- p_s + xinit first: they gate the first PE work, and the
            #      ACT HWDGE ring is otherwise empty here ----
            p_s = one.tile([128, MCHUNKS * G], F32R, name="p_s", tag="p_s")
            nc.scalar.dma_start(out=p_s, in_=pmat[:, :])
            x_cur = sml.tile([E, 4 * T], F32, name="x0", tag="xtile")
            nc.scalar.dma_start(out=x_cur[:, 0:T], in_=xinit[:, :])

            # ---- load constants / small inputs ----
            gb_s = one.tile([G, 1], F32, name="gb_s", tag="gb_s")
            nc.scalar.dma_start(out=gb_s, in_=gb[:, :])
            wkqt_s = one.tile([E, E], F32, name="wkqt_s", tag="wkqt_s")
            nc.scalar.dma_start(out=wkqt_s, in_=wkqt[:, :])
            wv_s = one.tile([E, E], F32, name="wv_s", tag="wv_s")
            nc.scalar.dma_start(out=wv_s, in_=wv[:, :])
            xstat_s = one.tile([E, D * 24], F32, name="xstat_s", tag="xstat_s")
            nc.scalar.dma_start(out=xstat_s, in_=xstat[:, :])
            m01_s = one.tile([4 * T, 4 * T], F32, name="m01_s", tag="m01_s")
            nc.scalar.dma_start(out=m01_s, in_=m01[:, :])
            gsel_s = one.tile([4 * T, T], F32, name="gsel_s", tag="gsel_s")
            nc.scalar.dma_start(out=gsel_s, in_=gsel[:, :])
            ones_s = one.tile([4 * T, 1], F32, name="ones_s", tag="ones_s")
            nc.vector.memset(ones_s, 1.0)

            # inf_mask = clip(log(action_mask)) — computed in a [32, 128]
            # lane-parallel layout, then gathered to [1, A] with one DMA
            lm32 = one.tile([32, 128], F32, name="lm32", tag="lm32")
            nc.scalar.dma_start(
                out=lm32, in_=amask.rearrange("o (p f) -> (o p) f", p=32))
            nc.scalar.activation(lm32, lm32, AF.Ln)
            nc.vector.tensor_scalar_max(lm32, lm32, FLOAT_MIN)
            nc.vector.tensor_scalar_min(lm32, lm32, FLOAT_MAX)
            lm_s = one.tile([1, A], F32, name="lm_s", tag="lm_s")
            nc.scalar.dma_start(out=lm_s, in_=lm32)

            # ---- tree scan, one step at a time (interleaved into the
            #      graph stream's PE program below) ----
            nc.vector.tensor_copy(x_cur[:, T:4 * T], xstat_s[:, 0:24])
            prev_final = one.tile([E, T], F32, name="prev_final",
                                  tag="prev_final")

            def tree_steps():
                # 4 yields per step: each PE sub-part's producers get a
                # ~1-2 graph-m-chunk head start, so the in-order PE queue
                # never stalls on the tree chain
                nonlocal x_cur
                for d in range(D):
                    # A = (Wk Wq^T) X  — folded kq product (host input)
                    a_p = tps.tile([E, 4 * T], F32, name="a_p", tag="tp")
                    nc.tensor.matmul(a_p, wkqt_s, x_cur, start=True, stop=True)
                    a_s = sml.tile([E, 4 * T], F32, name="a_s", tag="a_s")
                    nc.vector.tensor_copy(a_s, a_p)
                    v_p = tps.tile([4 * T, E], F32, name="v_p", tag="tp")
                    nc.tensor.matmul(v_p, x_cur, wv_s, start=True, stop=True)
                    v_s = sml.tile([4 * T, E], F32, name="v_s", tag="v_s")
                    nc.vector.tensor_copy(v_s, v_p)
                    yield
                    # S_T = X^T A  — S_T[k, q] = K[k]·Q[q]
                    s_p = tps.tile([4 * T, 4 * T], F32, name="s_p", tag="tp")
                    nc.tensor.matmul(s_p, x_cur, a_s, start=True, stop=True)
                    exp_s = sml.tile([4 * T, 4 * T], F32, name="exp_s",
                                     tag="exp_s")
                    nc.scalar.activation(exp_s, s_p, AF.Exp, scale=1.0 / E)
                    expm_s = sml.tile([4 * T, 4 * T], F32, name="expm_s",
                                      tag="expm_s")
                    nc.vector.tensor_mul(expm_s, exp_s, m01_s)
                    yield
                    rs_p = tps.tile([4 * T, 1], F32, name="rs_p", tag="tp")
                    nc.tensor.matmul(rs_p, expm_s, ones_s, start=True,
                                     stop=True)
                    rec_s = sml.tile([4 * T, 1], F32, name="rec_s",
                                     tag="rec_s")
                    nc.vector.reciprocal(rec_s, rs_p)
                    ou_p = tps.tile([4 * T, E], F32, name="ou_p", tag="tp")
                    nc.tensor.matmul(ou_p, expm_s, v_s, start=True, stop=True)
                    no_s = sml.tile([4 * T, E], F32, name="no_s", tag="no_s")
                    nc.vector.tensor_scalar_mul(no_s, ou_p, rec_s)
                    yield
                    nx_p = tps.tile([E, T], F32, name="nx_p", tag="tp")
                    nc.tensor.matmul(nx_p, no_s, gsel_s, start=True, stop=True)
                    if d + 1 < D:
                        x_nxt = sml.tile([E, 4 * T], F32, name="xn",
                                         tag="xtile")
                        nc.vector.tensor_copy(x_nxt[:, 0:T], nx_p)
                        nc.vector.tensor_copy(
                            x_nxt[:, T:4 * T],
                            xstat_s[:, (d + 1) * 24:(d + 2) * 24])
                        x_cur = x_nxt
                    else:
                        nc.vector.tensor_copy(prev_final, nx_p)
                    yield

            stepper = tree_steps()

            # ---- graph stream ----
            part_s = one.tile([G, NB_A + NB_B], F32, name="part_s",
                              tag="part_s")

            def stream_group(n0, nblocks, interleave):
                ps = []
                for j in range(nblocks):
                    pt = gps.tile([G, 512], F32, name=f"gp{j}", tag="gp")
                    ps.append(pt)
                width = nblocks * 512
                for mi in range(MCHUNKS):
                    lt = lnk.tile([128, width], F32R, name="lt", tag="lt")
                    nc.sync.dma_start(
                        out=lt,
                        in_=linkT[mi * 128:(mi + 1) * 128, n0:n0 + width])
                    lhs = p_s[:, mi * G:(mi + 1) * G]
                    for j in range(nblocks):
                        nc.tensor.matmul(
                            ps[j], lhs,
                            lt[:, j * 512:(j + 1) * 512],
                            start=(mi == 0), stop=(mi == MCHUNKS - 1))
                    if interleave:
                        next(stepper, None)
                return ps

            ps_a = stream_group(0, NB_A, True)
            for j in range(NB_A):
                rsc = sml.tile([G, 512], F32, name="rsc", tag="rsc")
                nc.scalar.activation(rsc, ps_a[j], AF.Relu, bias=gb_s,
                                     scale=1.0, accum_out=part_s[:, j:j + 1])
            # final-matmul weights: loaded here so the transfer hides under
            # the group-B stream instead of blocking the head of the kernel
            fcw_s = one.tile([G + E, A], F32R, name="fcw_s", tag="fcw_s")
            nc.scalar.dma_start(out=fcw_s, in_=fcw[:, :])
            ps_b = stream_group(NB_A * 512, NB_B, True)
            for _ in stepper:  # safety: finish any untraced tree steps
                pass
            for j in range(NB_B):
                rsc = sml.tile([G, 512], F32, name="rsc", tag="rsc")
                nc.scalar.activation(rsc, ps_b[j], AF.Relu, bias=gb_s,
                                     scale=1.0,
                                     accum_out=part_s[:, NB_A + j:NB_A + j + 1])

            pooled_f = one.tile([G, 1], F32, name="pooled_f", tag="pooled_f")
            nc.vector.reduce_sum(pooled_f, part_s, axis=AX.X)
            pooled_s = one.tile([G, 1], F32R, name="pooled_s", tag="pooled_s")
            nc.vector.tensor_scalar_mul(pooled_s, pooled_f, 1.0 / R)

            # tree_emb = sum_t prev_final
            temb_f = one.tile([E, 1], F32, name="temb_f", tag="temb_f")
            nc.vector.reduce_sum(temb_f, prev_final, axis=AX.X)
            temb_s = one.tile([E, 1], F32R, name="temb_s", tag="temb_s")
            nc.vector.tensor_copy(temb_s, temb_f)

            # logits = [pooled; tree_emb] @ fc_w + inf_mask
            feat_s = one.tile([G + E, 1], F32R, name="feat_s", tag="feat_s")
            nc.sync.dma_start(out=feat_s[0:G, :], in_=pooled_s)
            nc.scalar.dma_start(out=feat_s[G:G + E, :], in_=temb_s)
            out_s = one.tile([1, A], F32, name="out_s", tag="out_s")
            for j in range(A // 512):
                lg_p = tps.tile([1, 512], F32, name="lg_p", tag="tp")
                nc.tensor.matmul(lg_p, feat_s,
                                 fcw_s[:, j * 512:(j + 1) * 512],
                                 start=True, stop=True)
                nc.vector.tensor_add(out_s[:, j * 512:(j + 1) * 512], lg_p,
                                     lm_s[:, j * 512:(j + 1) * 512])
            nc.sync.dma_start(out=out[:, :], in_=out_s)

    _split_waits(nc)
    return nc


_NC_CACHE = None


def _get_nc():
    global _NC_CACHE
    if _NC_CACHE is None:
        _NC_CACHE = _build_nc()
    return _NC_CACHE


def _prepare_in_maps(inputs):
    link_mtx = np.ascontiguousarray(np.asarray(inputs["link_mtx"], np.float32))
    action_mask = np.asarray(inputs["action_mask"], np.float32)
    col = np.asarray(inputs["col_table"], np.float32)
    rel = np.asarray(inputs["rel_table"], np.float32)
    gw = np.ascontiguousarray(np.asarray(inputs["graph_weight"], np.float32))
    gb = np.asarray(inputs["graph_bias"], np.float32).reshape(G, 1)
    wk = np.ascontiguousarray(np.asarray(inputs["Wk"], np.float32))
    wq = np.ascontiguousarray(np.asarray(inputs["Wq"], np.float32))
    wv = np.ascontiguousarray(np.asarray(inputs["Wv"], np.float32))
    fcw = np.ascontiguousarray(np.asarray(inputs["fc_w"], np.float32))
    tid = np.asarray(inputs["table_ids"])
    lci = np.asarray(inputs["l_col_ids"])
    rci = np.asarray(inputs["r_col_ids"])

    gb = np.ascontiguousarray(gb)
    pmat = np.ascontiguousarray((rel @ gw).reshape(MCHUNKS, 128, G)
                                .transpose(1, 0, 2).reshape(128, MCHUNKS * G))
    wkqt = np.ascontiguousarray(wq @ wk.T)

    i = np.arange(4 * T)
    m01 = (i[:, None] % T == i[None, :] % T).astype(np.float32)
    gsel = (i[:, None] % T == np.arange(T)[None, :]).astype(np.float32)

    in_maps = []
    for c in range(N_CORES):
        linkT = np.ascontiguousarray(link_mtx[c].T)
        # host-side embedding gathers for the tree scan (layout prep)
        rtabs = rel[tid[c, :, 1:]]            # [T, D, E]
        lcols = col[lci[c]].mean(axis=-2)     # [T, D, E]
        rcols = col[rci[c]].mean(axis=-2)     # [T, D, E]
        xstat = np.empty((E, D, 24), np.float32)
        xstat[:, :, 0:8] = lcols.transpose(2, 1, 0)
        xstat[:, :, 8:16] = rcols.transpose(2, 1, 0)
        xstat[:, :, 16:24] = rtabs.transpose(2, 1, 0)
        xinit = np.ascontiguousarray(rel[tid[c, :, 0]].T)  # [E, T]
        in_maps.append({
            "linkt": linkT,
            "pmat": pmat,
            "gb": gb,
            "wkqt": wkqt,
            "wv": wv,
            "fcw": fcw,
            "amask": np.ascontiguousarray(action_mask[c:c + 1]),
            "xstat": np.ascontiguousarray(xstat.reshape(E, D * 24)),
            "xinit": xinit,
            "m01": m01,
            "gsel": gsel,
        })
    return in_maps


def _run(inputs, trace=False):
    nc = _get_nc()
    in_maps = _prepare_in_maps(inputs)
    res = run_bass_kernel_spmd(nc, in_maps, core_ids=list(range(N_CORES)),
                               trace=trace)
    out = np.stack([res.results[c]["out"].reshape(A) for c in range(N_CORES)])
    return out.astype(np.float32), res


def kernel(**inputs) -> np.ndarray:
    out, _ = _run(inputs, trace=False)
    return out


def kernel_traced(**inputs):
    out, res = _run(inputs, trace=True)
    return out, res
